# revision 7
# baseline (speedup 1.0000x reference)
"""CaLCS (soft-LCS) loss kernel for Trainium2, 8 NeuronCores, data-parallel over batch.

Problem (hardcoded shapes): batch [8, 512, 32000] f32 logits, docs [8, 512] int64.
  probs = softmax(batch, axis=2); p[b,i,j] = probs[b, i, docs[b,j]]
  D[i,j] = p*(1+D[i-1,j-1]) + (1-p)*max(D[i-1,j], D[i,j-1])   (anti-diagonal DP)
  loss = -log(mean_b min(D[511,511], 100) / 512)

Sharding: one batch element per core (SPMD, same program). Per core the device:
  1. streams its [512, 32000] logit rows, computing exp + row-sums on ACT
     (no max subtraction needed: inputs are randn, exp is safe in fp32)
  2. normalizes the host-pre-sliced logit columns into p = exp(x)/Z and q = 1-p
  3. writes p and q to DRAM in a skewed (diagonal-major, partition-lagged)
     layout so each DP step reads its operands as regular access patterns
  4. runs the wavefront DP. Lanes are laid out free-major: DP row i lives at
     [partition i//4, lane i%4]. Hardware cannot shift across partitions on
     compute engines, so the grid is *time-skewed*: partition p runs LAG=1
     instruction-steps behind partition p-1. The cross-partition boundary
     value then has 2 steps of slack and is produced off the critical path
     by a PE shift-matmul into PSUM + an ACT copy into the guard column.
     The DVE critical chain is 3 small ops per step (max, mul, add).
Host: gathers the 8 clamped D values, returns -log(mean/512).
"""

import numpy as np

import bass_rust
import concourse.bass as bass
import concourse.tile as tile
import concourse.mybir as mybir
from concourse import bass_utils

# ---- problem constants (hardcoded per contract) ----
B = 8
R = 512          # generation steps (rows of DP grid)
V = 32000        # vocab
C = 512          # doc length (cols of DP grid)
CLAMP = 100.0
P = 128          # SBUF partitions
NGRP = R // P    # 4 row groups
VCHUNK = 4000
NCHUNK = V // VCHUNK
ND = R + C - 1     # 1023 diagonals
LAG = 1            # instruction-step lag per partition
NSTEP = ND + (P - 1) * LAG   # 1150 DP instruction steps
DW = 1280          # skew buffer row width (u = j + i + p <= 1149)
SFLAT = P * 4 * DW
NRING = 4

F32 = mybir.dt.float32
ALU = mybir.AluOpType
ACTF = mybir.ActivationFunctionType


def _patched_drain_and_barrier(self, tick_clock, wait_clock):
    """Split the kernel-tail drain's sem waits across multiple drain
    instructions — core_v3 codegen rejects multi-wait CTRL instructions."""
    from concourse.tile import ScopedClock

    nc = self.nc
    probe = nc.sync.drain()
    wait_clock.add_sem_waits(probe.ins, ScopedClock({None: tick_clock.global_clock}))
    waits = list(probe.ins.sync_info.on_wait) if probe.ins.sync_info else []
    if len(waits) > 1:
        probe.ins.sync_info = bass_rust.SyncInfo(on_wait=waits[:1], on_update=[])
        for i in range(1, len(waits)):
            d = nc.sync.drain()
            d.ins.sync_info = bass_rust.SyncInfo(on_wait=[waits[i]], on_update=[])
    nc.all_engine_barrier()
    popped = nc._tile_sem_poison_stack.pop()
    assert popped is self._sem_poison
    nc.clear_and_free_semaphores(list(self.sems.allocated().values()))
    nc.all_engine_barrier()


tile.TileContext._drain_and_barrier = _patched_drain_and_barrier


def _split_multi_waits(nc: bass.Bass):
    """Walrus codegen for TRN2 accepts at most one sem wait per instruction.
    Hoist extra waits into same-engine NoOp/Drain instructions inserted
    immediately before the offending instruction."""
    n_split = 0
    for fn in nc.m.functions:
        for blk in fn.blocks:
            il = blk.instructions
            i = 0
            while i < len(il):
                inst = il[i]
                si = inst.sync_info
                if si is not None and len(si.on_wait) > 1:
                    waits = list(si.on_wait)
                    inst.sync_info = bass_rust.SyncInfo(
                        on_wait=[waits[0]], on_update=list(si.on_update)
                    )
                    for k, w in enumerate(waits[1:]):
                        if inst.engine == mybir.EngineType.PE:
                            filler = mybir.InstDrain(
                                name=f"wsplit-{inst.name}-{k}", engine=inst.engine,
                                sync_info=bass_rust.SyncInfo(on_wait=[w], on_update=[]),
                            )
                        else:
                            filler = mybir.InstNoOp(
                                name=f"wsplit-{inst.name}-{k}", engine=inst.engine,
                                sync_info=bass_rust.SyncInfo(on_wait=[w], on_update=[]),
                            )
                        il.insert(i, filler)
                        i += 1
                        n_split += 1
                i += 1
    return n_split


def _skew_write_ap(dram_handle, grp: int):
    """DRAM AP scattering a [128, 512] row-group tile of the [512, 512]
    p/q matrix into the skewed flat buffer.

    Lane map: DP row i -> (partition p = i//4, lane g = i%4).
    Skew: S[p, g, u] = mat[i, j] at u = j + i + LAG*p.
    For source partition ph = 4a + b (row i = 128*grp + ph): p = 32*grp + a,
    g = b, so dest = p*4*DW + g*DW + j + i + p
                   = grp*(128*DW + 160) + a*(4*DW + 5) + b*(DW + 1) + j.
    """
    base = dram_handle[:]
    return bass.AP(
        tensor=base.tensor,
        offset=(128 * DW + 160) * grp,
        ap=[[4 * DW + 5, 32], [DW + 1, 4], [1, C]],
    )


def build_nc() -> bass.Bass:
    nc = bass.Bass(trn_type="TRN2")
    x = nc.dram_tensor("x", [R, V], F32, kind="ExternalInput")
    cols = nc.dram_tensor("cols", [R, C], F32, kind="ExternalInput")
    out = nc.dram_tensor("out", [1, 1], F32, kind="ExternalOutput")
    s_p = nc.dram_tensor("skew_p", [SFLAT], F32, kind="Internal")
    s_q = nc.dram_tensor("skew_q", [SFLAT], F32, kind="Internal")

    # shift matrix: (Wsh.T @ v)[p] = v[p-1], row 0 -> 0
    wnp = np.zeros((P, P), dtype=np.float32)
    wnp[np.arange(P - 1), np.arange(1, P)] = 1.0
    w_dram = nc.inline_tensor(wnp, name="wshift")

    with tile.TileContext(nc) as tc:
        with (
            tc.tile_pool(name="chunks", bufs=3) as chunks,
            tc.tile_pool(name="singles", bufs=1) as singles,
            tc.tile_pool(name="work", bufs=2) as work,
            tc.tile_pool(name="psum", bufs=1, space="PSUM") as psp,
        ):
            # --- zero-fill the skew scratch buffers in DRAM ---
            zt = singles.tile([P, 4 * DW], F32, tag="zeros")
            nc.vector.memset(zt[:, :], 0.0)
            zview_p = bass.AP(tensor=s_p[:].tensor, offset=0,
                              ap=[[4 * DW, P], [1, 4 * DW]])
            zview_q = bass.AP(tensor=s_q[:].tensor, offset=0,
                              ap=[[4 * DW, P], [1, 4 * DW]])
            nc.gpsimd.dma_start(out=zview_p, in_=zt[:, :])
            nc.gpsimd.dma_start(out=zview_q, in_=zt[:, :])

            wsb = singles.tile([P, P], F32, tag="wsb")
            nc.gpsimd.dma_start(out=wsb[:, :], in_=w_dram[:, :])

            # --- phase 1: stream logits, exp + row-sum on ACT; build p/q skew ---
            for grp in range(NGRP):
                r0 = grp * P
                sums = singles.tile([P, NCHUNK], F32, tag=f"sums{grp}",
                                    name=f"sums{grp}")
                for k in range(NCHUNK):
                    t = chunks.tile([P, VCHUNK], F32, tag="stream", name="stream_t")
                    nc.sync.dma_start(
                        out=t[:, :],
                        in_=x[r0:r0 + P, k * VCHUNK:(k + 1) * VCHUNK],
                    )
                    nc.scalar.activation(
                        out=t[:, :], in_=t[:, :], func=ACTF.Exp,
                        accum_out=sums[:, k:k + 1],
                    )
                z = singles.tile([P, 1], F32, tag=f"z{grp}", name=f"z{grp}")
                nc.vector.tensor_reduce(
                    out=z[:, :], in_=sums[:, :], axis=mybir.AxisListType.X,
                    op=ALU.add,
                )
                rcp = singles.tile([P, 1], F32, tag=f"rcp{grp}", name=f"rcp{grp}")
                nc.vector.reciprocal(out=rcp[:, :], in_=z[:, :])
                nrcp = singles.tile([P, 1], F32, tag=f"nrcp{grp}", name=f"nrcp{grp}")
                nc.vector.tensor_scalar(
                    out=nrcp[:, :], in0=rcp[:, :], scalar1=-1.0, scalar2=None,
                    op0=ALU.mult,
                )
                ct = work.tile([P, C], F32, tag="ct", name="ct")
                nc.sync.dma_start(out=ct[:, :], in_=cols[r0:r0 + P, :])
                nc.scalar.activation(out=ct[:, :], in_=ct[:, :], func=ACTF.Exp)
                qt = work.tile([P, C], F32, tag="qt", name="qt")
                # q = 1 - exp(x)/Z = (expx * -1/Z) + 1
                nc.vector.tensor_scalar(
                    out=qt[:, :], in0=ct[:, :], scalar1=nrcp[:, :], scalar2=1.0,
                    op0=ALU.mult, op1=ALU.add,
                )
                # p = exp(x)/Z  (in place)
                nc.vector.tensor_scalar(
                    out=ct[:, :], in0=ct[:, :], scalar1=rcp[:, :], scalar2=None,
                    op0=ALU.mult,
                )
                nc.gpsimd.dma_start(out=_skew_write_ap(s_p, grp), in_=ct[:, :])
                nc.gpsimd.dma_start(out=_skew_write_ap(s_q, grp), in_=qt[:, :])

            # --- load skewed p/q into SBUF ---
            p_sb = singles.tile([P, 4, DW], F32, tag="p_sb")
            q_sb = singles.tile([P, 4, DW], F32, tag="q_sb")
            pview = bass.AP(tensor=s_p[:].tensor, offset=0,
                            ap=[[4 * DW, P], [DW, 4], [1, DW]])
            qview = bass.AP(tensor=s_q[:].tensor, offset=0,
                            ap=[[4 * DW, P], [DW, 4], [1, DW]])
            nc.sync.dma_start(out=p_sb[:, :, :], in_=pview)
            nc.sync.dma_start(out=q_sb[:, :, :], in_=qview)

            # --- phase 2: time-skewed wavefront DP ---
            # N tiles [128, 5]: col 0 = guard (cross-partition boundary value),
            # cols 1..4 = lanes g = 0..3. At instruction step t, partition p
            # computes DP anti-diagonal s = t - p for its 4 rows.
            nring = [singles.tile([P, 5], F32, tag=f"nring{j}", name=f"nring{j}")
                     for j in range(NRING)]
            for j in range(NRING):
                nc.vector.memset(nring[j][:, :], 0.0)
            wring = [singles.tile([P, 4], F32, tag=f"wring{j}", name=f"wring{j}")
                     for j in range(3)]
            psring = [psp.tile([P, 1], F32, tag=f"ps{j}", name=f"ps{j}")
                      for j in range(2)]
            mt = singles.tile([P, 4], F32, tag="mt")
            wtmp = singles.tile([P, 4], F32, tag="wtmp")
            ztile = singles.tile([P, 4], F32, tag="ztile")

            for t in range(NSTEP):
                np2 = nring[(t - 2) % NRING]
                np1 = nring[(t - 1) % NRING]
                nd = nring[t % NRING]
                w = wring[t % 3]
                ps = psring[t % 2]
                # guard producer (2-step slack): PE shifts the boundary
                # column down one partition; ACT lands it in nd's guard col.
                nc.tensor.matmul(out=ps[:, :], lhsT=wsb[:, :], rhs=np1[:, 4:5])
                nc.scalar.copy(out=nd[:, 0:1], in_=ps[:, :])
                # off critical path: w = (D[i-1,j-1] + 1) * p = D*p + p
                # (Pool codegen only supports TensorTensor add/mult)
                nc.gpsimd.tensor_tensor(
                    out=wtmp[:, :], in0=np2[:, 0:4], in1=p_sb[:, :, t], op=ALU.mult,
                )
                nc.gpsimd.tensor_tensor(
                    out=w[:, :], in0=wtmp[:, :], in1=p_sb[:, :, t], op=ALU.add,
                )
                # DVE critical chain: m = max(D[i-1,j], D[i,j-1])
                nc.vector.tensor_tensor(
                    out=mt[:, :], in0=np1[:, 0:4], in1=np1[:, 1:5], op=ALU.max,
                )
                nc.vector.tensor_tensor(
                    out=ztile[:, :], in0=q_sb[:, :, t], in1=mt[:, :], op=ALU.mult,
                )
                nc.vector.tensor_tensor(
                    out=nd[:, 1:5], in0=w[:, :], in1=ztile[:, :], op=ALU.add,
                )

            # --- epilogue: D[511,511] = partition 127 lane 3 at step NSTEP-1.
            # Compute engines cannot address partition 127 directly; DMA the
            # raw value out and clamp on the host.
            last = nring[(NSTEP - 1) % NRING]
            nc.sync.dma_start(out=out[:, :], in_=last[P - 1:P, 4:5])

    _split_multi_waits(nc)
    return nc


def kernel(batch: np.ndarray, docs: np.ndarray) -> np.ndarray:
    batch = np.ascontiguousarray(np.asarray(batch, dtype=np.float32))
    docs = np.asarray(docs)
    assert batch.shape == (B, R, V) and docs.shape == (B, C)

    nc = build_nc()
    in_maps = []
    for b in range(B):
        cols_b = np.ascontiguousarray(batch[b][:, docs[b].astype(np.int64)])
        in_maps.append({"x": batch[b], "cols": cols_b})

    res = bass_utils.run_bass_kernel_spmd(nc, in_maps, core_ids=list(range(B)))
    d_vals = np.array(
        [res.results[b]["out"][0, 0] for b in range(B)], dtype=np.float64
    )
    d_vals = np.minimum(d_vals, CLAMP)
    loss = -np.log(d_vals.mean() / float(C))
    return np.float32(loss)


# revision 10
# speedup vs baseline: 94.7763x; 94.7763x over previous
"""CaLCS (soft-LCS) loss kernel for Trainium2, 8 NeuronCores, data-parallel over batch.

Problem (hardcoded shapes): batch [8, 512, 32000] f32 logits, docs [8, 512] int64.
  probs = softmax(batch, axis=2); p[b,i,j] = probs[b, i, docs[b,j]]
  D[i,j] = p*(1+D[i-1,j-1]) + (1-p)*max(D[i-1,j], D[i,j-1])
  loss = -log(mean_b min(D[511,511], 100) / 512)

Sharding: one batch element per core (SPMD, same program).

This target executes ~one instruction per ~40us regardless of operand size,
so the design minimizes instruction count:

Phase 1 (per core, ~75 instructions): stream the [512, 32000] logits in
  8 big chunks, exp + row-sum on ACT (randn logits: exp is fp32-safe without
  max subtraction); normalize the host-pre-sliced logit columns into
  p = exp(x)/Z, q = 1-p; compute R = prefix-prod(q) (one scan per row
  group), invR = 1/R, pinv = p*invR; pack per-row vectors
  [pinv_i, invRshift_i, R_i] into DRAM.

Phase 2 (~4 instructions per DP row): the row recurrence
    a_j = K_j + q_j * max(b_j, a_{j-1}),   K_j = p_j * (1 + b_{j-1})
  (a = D row i, b = D row i-1) normalized by alpha_j = a_j / R_j becomes a
  pure (max,+) scan:
    alpha_j = max(v_j, alpha_{j-1}) + Khat_j,
    v_j = b_j / R_{j-1},  Khat_j = (b_{j-1} + 1) * p_j / R_j
  which is exactly one hardware tensor_tensor_scan (op0=max, op1=add).
  Per row: Khat (scalar_tensor_tensor), v (tensor_tensor), the scan, and
  a = alpha * R (tensor_tensor), all [1, 512] on partition 0, plus one
  staging DMA per 8 rows for the packed per-row constants.

Host: gathers the 8 clamped D values, returns -log(mean/512).
"""

import numpy as np

import bass_rust
import concourse.bass as bass
import concourse.tile as tile
import concourse.mybir as mybir
from concourse import bass_utils

# ---- problem constants (hardcoded per contract) ----
B = 8
R = 512          # generation steps (rows of DP grid)
V = 32000        # vocab
C = 512          # doc length (cols of DP grid)
CLAMP = 100.0
P = 128          # SBUF partitions
NGRP = R // P    # 4 row groups
VCHUNK = 16000
NCHUNK = V // VCHUNK   # 2 chunks per row group
FW = C           # packed field width
ROWSTRIDE = 3 * FW
BLK = 8          # rows staged per DMA

F32 = mybir.dt.float32
ALU = mybir.AluOpType
ACTF = mybir.ActivationFunctionType


def _patched_drain_and_barrier(self, tick_clock, wait_clock):
    """Split the kernel-tail drain's sem waits across multiple drain
    instructions — core_v3 codegen rejects multi-wait CTRL instructions."""
    from concourse.tile import ScopedClock

    nc = self.nc
    probe = nc.sync.drain()
    wait_clock.add_sem_waits(probe.ins, ScopedClock({None: tick_clock.global_clock}))
    waits = list(probe.ins.sync_info.on_wait) if probe.ins.sync_info else []
    if len(waits) > 1:
        probe.ins.sync_info = bass_rust.SyncInfo(on_wait=waits[:1], on_update=[])
        for i in range(1, len(waits)):
            d = nc.sync.drain()
            d.ins.sync_info = bass_rust.SyncInfo(on_wait=[waits[i]], on_update=[])
    nc.all_engine_barrier()
    popped = nc._tile_sem_poison_stack.pop()
    assert popped is self._sem_poison
    nc.clear_and_free_semaphores(list(self.sems.allocated().values()))
    nc.all_engine_barrier()


tile.TileContext._drain_and_barrier = _patched_drain_and_barrier


def _split_multi_waits(nc: bass.Bass):
    """Walrus codegen for TRN2 accepts at most one sem wait per instruction.
    Hoist extra waits into same-engine NoOp/Drain instructions inserted
    immediately before the offending instruction."""
    n_split = 0
    for fn in nc.m.functions:
        for blk in fn.blocks:
            il = blk.instructions
            i = 0
            while i < len(il):
                inst = il[i]
                si = inst.sync_info
                if si is not None and len(si.on_wait) > 1:
                    waits = list(si.on_wait)
                    inst.sync_info = bass_rust.SyncInfo(
                        on_wait=[waits[0]], on_update=list(si.on_update)
                    )
                    for k, w in enumerate(waits[1:]):
                        if inst.engine == mybir.EngineType.PE:
                            filler = mybir.InstDrain(
                                name=f"wsplit-{inst.name}-{k}", engine=inst.engine,
                                sync_info=bass_rust.SyncInfo(on_wait=[w], on_update=[]),
                            )
                        else:
                            filler = mybir.InstNoOp(
                                name=f"wsplit-{inst.name}-{k}", engine=inst.engine,
                                sync_info=bass_rust.SyncInfo(on_wait=[w], on_update=[]),
                            )
                        il.insert(i, filler)
                        i += 1
                        n_split += 1
                i += 1
    return n_split


def build_nc(timing_reps: int = 0, *, dp_rows: int = R,
             do_phase1: bool = True, do_dp: bool = True) -> bass.Bass:
    """timing_reps=0: normal build (external inputs). timing_reps=K>0:
    inputs are Internal DRAM (zero-filled on device) and the whole body is
    repeated K times with barriers between reps, so wall-clock differences
    between rep counts isolate per-invocation device time."""
    nc = bass.Bass(trn_type="TRN2")
    kind = "Internal" if timing_reps else "ExternalInput"
    x = nc.dram_tensor("x", [R, V], F32, kind=kind)
    cols = nc.dram_tensor("cols", [R, C], F32, kind=kind)
    out = nc.dram_tensor("out", [1, 1], F32, kind="ExternalOutput")
    packed = nc.dram_tensor("packed", [R * ROWSTRIDE], F32, kind="Internal")

    with tile.TileContext(nc) as tc:
        with tc.tile_pool(name="keep", bufs=1) as keep:
            if timing_reps:
                zx = keep.tile([P, VCHUNK], F32, tag="zx")
                nc.vector.memset(zx[:, :], 0.0)
                for grp in range(NGRP):
                    for k in range(NCHUNK):
                        nc.gpsimd.dma_start(
                            out=x[grp * P:(grp + 1) * P,
                                  k * VCHUNK:(k + 1) * VCHUNK],
                            in_=zx[:, :])
                    nc.gpsimd.dma_start(
                        out=cols[grp * P:(grp + 1) * P, :], in_=zx[:, :C])
                tc.strict_bb_all_engine_barrier()

            def emit_body():
                # ---------- phase 1 ----------
                if do_phase1:
                    with (
                        tc.tile_pool(name="chunks", bufs=2) as chunks,
                        tc.tile_pool(name="p1", bufs=1) as p1,
                    ):
                        ones = p1.tile([P, C], F32, tag="ones")
                        nc.vector.memset(ones[:, :], 1.0)
                        for grp in range(NGRP):
                            r0 = grp * P
                            sums = p1.tile([P, NCHUNK], F32, tag="sums",
                                           name="sums")
                            for k in range(NCHUNK):
                                t = chunks.tile([P, VCHUNK], F32, tag="stream",
                                                name="stream_t")
                                nc.sync.dma_start(
                                    out=t[:, :],
                                    in_=x[r0:r0 + P,
                                          k * VCHUNK:(k + 1) * VCHUNK],
                                )
                                nc.scalar.activation(
                                    out=t[:, :], in_=t[:, :], func=ACTF.Exp,
                                    accum_out=sums[:, k:k + 1],
                                )
                            z = p1.tile([P, 1], F32, tag="z", name="zz")
                            nc.vector.tensor_reduce(
                                out=z[:, :], in_=sums[:, :],
                                axis=mybir.AxisListType.X, op=ALU.add,
                            )
                            rcp = p1.tile([P, 1], F32, tag="rcp", name="rcp")
                            nc.vector.reciprocal(out=rcp[:, :], in_=z[:, :])
                            nrcp = p1.tile([P, 1], F32, tag="nrcp", name="nrcp")
                            nc.vector.tensor_scalar(
                                out=nrcp[:, :], in0=rcp[:, :], scalar1=-1.0,
                                scalar2=None, op0=ALU.mult,
                            )
                            ct = p1.tile([P, C], F32, tag="ct", name="ct")
                            nc.sync.dma_start(out=ct[:, :],
                                              in_=cols[r0:r0 + P, :])
                            nc.scalar.activation(out=ct[:, :], in_=ct[:, :],
                                                 func=ACTF.Exp)
                            qt = p1.tile([P, C], F32, tag="qt", name="qt")
                            # q = 1 - exp(x)/Z
                            nc.vector.tensor_scalar(
                                out=qt[:, :], in0=ct[:, :], scalar1=nrcp[:, :],
                                scalar2=1.0, op0=ALU.mult, op1=ALU.add,
                            )
                            # p = exp(x)/Z  (in place)
                            nc.vector.tensor_scalar(
                                out=ct[:, :], in0=ct[:, :], scalar1=rcp[:, :],
                                scalar2=None, op0=ALU.mult,
                            )
                            # Rext[:, 0] = 1; Rext[:, 1+j] = prod_{t<=j} q_t
                            rext = p1.tile([P, C + 1], F32, tag="rext",
                                           name="rext")
                            nc.vector.memset(rext[:, 0:1], 1.0)
                            nc.vector.tensor_tensor_scan(
                                out=rext[:, 1:C + 1], data0=qt[:, :],
                                data1=ones[:, :], initial=1.0,
                                op0=ALU.mult, op1=ALU.mult,
                            )
                            irext = p1.tile([P, C + 1], F32, tag="irext",
                                            name="irext")
                            nc.vector.reciprocal(out=irext[:, :],
                                                 in_=rext[:, :])
                            # pinv = p * invR   (in place over ct)
                            nc.vector.tensor_tensor(
                                out=ct[:, :], in0=ct[:, :],
                                in1=irext[:, 1:C + 1], op=ALU.mult,
                            )
                            # pack [pinv, invRsh, R] per row i = 128*grp + ph:
                            # packed[i*ROWSTRIDE + f*FW + j]
                            for f, src in (
                                (0, ct[:, :]),            # pinv
                                (1, irext[:, 0:C]),       # 1/R_{j-1}
                                (2, rext[:, 1:C + 1]),    # R_j
                            ):
                                dst = bass.AP(
                                    tensor=packed[:].tensor,
                                    offset=r0 * ROWSTRIDE + f * FW,
                                    ap=[[ROWSTRIDE, P], [1, FW]],
                                )
                                nc.sync.dma_start(out=dst, in_=src)

                # ---------- phase 2: row scans on partition 0 ----------
                if do_dp:
                    with tc.tile_pool(name="dp", bufs=1) as dp, \
                         tc.tile_pool(name="stage", bufs=2) as stpool:
                        a0 = dp.tile([1, C + 1], F32, tag="a0")
                        a1 = dp.tile([1, C + 1], F32, tag="a1")
                        nc.vector.memset(a0[:, :], 0.0)
                        nc.vector.memset(a1[:, :], 0.0)
                        khat = dp.tile([1, C], F32, tag="khat")
                        vv = dp.tile([1, C], F32, tag="vv")
                        alpha = dp.tile([1, C], F32, tag="alpha")
                        abufs = [a0, a1]
                        stage = None
                        for i in range(dp_rows):
                            if i % BLK == 0:
                                stage = stpool.tile([1, BLK * ROWSTRIDE], F32,
                                                    tag="stage", name="stage")
                                src = bass.AP(
                                    tensor=packed[:].tensor,
                                    offset=i * ROWSTRIDE,
                                    ap=[[1, BLK * ROWSTRIDE]],
                                )
                                nc.sync.dma_start(out=stage[0:1, :], in_=src)
                            o = (i % BLK) * ROWSTRIDE
                            pinv_r = stage[0:1, o:o + FW]
                            invrsh_r = stage[0:1, o + FW:o + 2 * FW]
                            r_r = stage[0:1, o + 2 * FW:o + 3 * FW]
                            aprev = abufs[i % 2]
                            acur = abufs[(i + 1) % 2]
                            # Khat = (b_{j-1} + 1) * pinv
                            nc.vector.scalar_tensor_tensor(
                                out=khat[:, :], in0=aprev[0:1, 0:C],
                                scalar=1.0, in1=pinv_r,
                                op0=ALU.add, op1=ALU.mult,
                            )
                            # v = b_j / R_{j-1}
                            nc.vector.tensor_tensor(
                                out=vv[:, :], in0=aprev[0:1, 1:C + 1],
                                in1=invrsh_r, op=ALU.mult,
                            )
                            # alpha_j = max(v_j, alpha_{j-1}) + Khat_j
                            nc.vector.tensor_tensor_scan(
                                out=alpha[:, :], data0=vv[:, :],
                                data1=khat[:, :], initial=0.0,
                                op0=ALU.max, op1=ALU.add,
                            )
                            # a_j = alpha_j * R_j
                            nc.vector.tensor_tensor(
                                out=acur[0:1, 1:C + 1], in0=alpha[:, :],
                                in1=r_r, op=ALU.mult,
                            )
                        final = abufs[dp_rows % 2]
                        nc.sync.dma_start(out=out[:, :],
                                          in_=final[0:1, C:C + 1])

            for _rep in range(max(1, timing_reps)):
                if _rep:
                    tc.strict_bb_all_engine_barrier()
                emit_body()

    _split_multi_waits(nc)
    return nc


def kernel(batch: np.ndarray, docs: np.ndarray) -> np.ndarray:
    batch = np.ascontiguousarray(np.asarray(batch, dtype=np.float32))
    docs = np.asarray(docs)
    assert batch.shape == (B, R, V) and docs.shape == (B, C)

    nc = build_nc()
    in_maps = []
    for b in range(B):
        cols_b = np.ascontiguousarray(batch[b][:, docs[b].astype(np.int64)])
        in_maps.append({"x": batch[b], "cols": cols_b})

    res = bass_utils.run_bass_kernel_spmd(nc, in_maps, core_ids=list(range(B)))
    d_vals = np.array(
        [res.results[b]["out"][0, 0] for b in range(B)], dtype=np.float64
    )
    d_vals = np.minimum(d_vals, CLAMP)
    loss = -np.log(d_vals.mean() / float(C))
    return np.float32(loss)
